# revision 37
# baseline (speedup 1.0000x reference)
"""Trainium2 Bass kernel for nn_AttentionModule (Bahdanau-style attention).

Reference computation (S=512, B=64, H=1024, F=2H):
    cat    = concat([hidden bcast to (S,B,H), encoder_states], -1)      [S,B,2H]
    scores = tanh(cat @ W_attn.T + b_attn) @ W_attn2.T + b_attn2        [S,B,1]
    attn   = softmax(scores[..., 0].T, axis=-1)                         [B,S]
    applied= einsum("bs,sbh->bh", attn, encoder_states)                 [B,H]
    out    = tanh(concat([decoder_out, applied], -1) @ W_comb.T + b_comb)

Sharding: data-parallel over B across 8 cores (8 batch rows per core).

Per-core structure (v4):
  - Main matmul z[f,s] = sum_h W1e[f,h] enc[h,s] runs in fp8 e4m3 with
    MatmulPerfMode.DoubleRow: both operands supply 2 contraction elements
    per partition (pairs of 128-deep h chunks), 0.5 PE cycles per output
    row. Weights are pre-scaled by 2^9 on the host (fp8 dynamic range);
    the tanh activation applies scale=2^-9.
  - The hidden-side preamble hidbT[f, b] = (hidden @ W1h.T)^T also runs
    in fp8 DoubleRow; bias b_attn and the 2^-9 descale fuse into one DVE
    tensor_scalar when draining PSUM. Validated in fp64 simulation:
    fp8 main+preamble keeps rel err ~9e-3 < 2e-2. Quantizing the scores
    matmul or the applied-sum encoder copy to fp8 does NOT fit the error
    budget, so those stay bf16.
  - DMA is one serial ~360GB/s queue with ~0.6us fixed cost per
    descriptor, so every host tensor is pre-swizzled to load as a single
    contiguous [128, N] DMA; the two fp8 weight tensors are split into
    quarters and interleaved so PE starts ~3us into the kernel; encoder
    tiles prefetch staggered (enc8 two rows ahead, enc16 one row ahead
    of use). Outputs ship in SBUF layout and the host unswizzles.
  - scores via bf16 PE matmul contracting f (lhsT = replicated W_attn2).
  - softmax on one partition, WITHOUT the max-subtraction pass
    (|scores| < 46 so fp32 exp cannot overflow); normalization folds
    into a K=1 PE matmul: lhsT = (1/sumexp) replicated to 128 columns,
    rhs = unnormalized exp row -> PSUM [128, S] = normalized attention
    on all 128 partitions, drained to SBUF bf16 on DVE.
  - applied^T[h, b] via DVE TensorTensor(mult, bf16 2x mode) +
    TensorReduce per (b, h-chunk). The last row's chunks also drain to
    bf16 on idle GPSIMD so the combine can start per-chunk.
  - Final combine in out^T layout: stationary Wc chunk [128f, 128h],
    moving cat^T [128f, 8b] -> PSUM [128h, 8b]; tanh + b_comb bias is a
    single per-partition-bias activation per h-chunk.

Known pitfalls baked into this implementation:
  - bf16/fp8 host arrays with tiny rows get corrupted on the host->device
    path, so small tensors ship as fp32 and are cast on device.
  - multi-dim rearrange DMAs need >=1KB contiguous inner blocks (smaller
    blocks corrupt data on HW) -> host pre-swizzles everything else.
  - 16/32-bit matmul operand mixing is rejected by the compiler.
  - tensor_tensor_reduce (the fused DVE ISA op) and DVE ISA ops reading
    PSUM both hard-crash the device (NRT_EXEC_UNIT_UNRECOVERABLE):
    stick to plain TensorTensor/TensorReduce on SBUF operands.
  - each DMA costs ~0.6us on the serial HWDGE descriptor path, so small
    tensors must be coalesced into single contiguous loads.
"""

import numpy as np

S, B, H = 512, 64, 1024
F = 2 * H
NCORES = 8
BL = B // NCORES          # 8 batch rows per core
KH = H // 128             # 8 contraction chunks over H
KP = KH // 2              # 4 fp8 DoubleRow chunk pairs
KF = F // 128             # 16 feature tiles
KO = H // 128             # 8 output-H chunks
WSCALE = 2.0 ** 9         # host pre-scale on fp8 W1, undone on device
NQ = 8                    # weight-load chunks (interleaved w1h/w1e)

_CACHE = {}


def _build(num_devices=NCORES, stage=6, nb=BL):
    # stage (debug bisection): 1=preamble, 2=+main/tanh, 3=+scores/softmax,
    # 4=+applied, 5=+all-b loop, 6=full (default). nb: batch rows in loop.
    from contextlib import ExitStack

    import concourse.tile as tile
    from concourse import bacc, mybir

    f32 = mybir.dt.float32
    bf16 = mybir.dt.bfloat16
    fp8 = mybir.dt.float8e4
    AF = mybir.ActivationFunctionType
    ALU = mybir.AluOpType
    AX = mybir.AxisListType
    PM = mybir.MatmulPerfMode

    nc = bacc.Bacc("TRN2", target_bir_lowering=False, debug=False,
                   num_devices=num_devices)

    # encoder: per-b partition-major contiguous [BL, 128, KH*S]
    enc8_d = nc.dram_tensor("enc8", [BL, 128, KH * S], fp8,
                            kind="ExternalInput").ap()
    enc16_d = nc.dram_tensor("enc16", [BL, 128, KH * S], bf16,
                             kind="ExternalInput").ap()
    # W1 halves, fp8, ft-major: [128, KF, KH*128] flattened
    w1e8_d = nc.dram_tensor("w1e8", [128, KF * KH * 128], fp8,
                            kind="ExternalInput").ap()
    w1h8_d = nc.dram_tensor("w1h8", [128, KF * KH * 128], fp8,
                            kind="ExternalInput").ap()
    wct = nc.dram_tensor("wct", [F, H], bf16, kind="ExternalInput").ap()
    # host-swizzled [128, K*BL] fp32 smalls
    hidT_d = nc.dram_tensor("hidTs", [128, KH * BL], f32,
                            kind="ExternalInput").ap()
    decT_d = nc.dram_tensor("decTs", [128, KH * BL], f32,
                            kind="ExternalInput").ap()
    w2rep_d = nc.dram_tensor("w2reps", [128, KF * BL], f32,
                             kind="ExternalInput").ap()
    b_attnT_d = nc.dram_tensor("b_attnT", [128, KF], f32,
                               kind="ExternalInput").ap()
    b_combT_d = nc.dram_tensor("b_combT", [128, KO], f32,
                               kind="ExternalInput").ap()
    # outputs in SBUF layout; host unswizzles
    outT_d = nc.dram_tensor("outT", [128, KO * BL], f32,
                            kind="ExternalOutput").ap()
    appT_d = nc.dram_tensor("appliedT", [128, KH * BL], f32,
                            kind="ExternalOutput").ap()

    CHW = KH * 128            # columns per ft chunk of w1 tensors
    QW = KF * CHW // NQ       # columns per w1 quarter

    with tile.TileContext(nc) as tc:
        with ExitStack() as ctx:
            consts = ctx.enter_context(tc.tile_pool(name="consts", bufs=1))
            enc8_pool = ctx.enter_context(tc.tile_pool(name="enc8", bufs=3))
            enc16_pool = ctx.enter_context(tc.tile_pool(name="enc16", bufs=3))
            th_pool = ctx.enter_context(tc.tile_pool(name="th", bufs=8))
            scr_pool = ctx.enter_context(tc.tile_pool(name="scr", bufs=4))
            abc_pool = ctx.enter_context(tc.tile_pool(name="abc", bufs=2))
            small_pool = ctx.enter_context(tc.tile_pool(name="small", bufs=4))
            # PSUM: pT 4 banks + psc 2 + abc 1 + pre 1 = 8. The preamble gets
            # its own bank so the main loop's pT ring never waits on it;
            # combine borrows the pT tag (sub-sliced, runs after the loop).
            # abc needs only 1 buf: row b's ttr chain finishes well before
            # row b+1's softmax produces the next broadcast.
            psT_pool = ctx.enter_context(
                tc.tile_pool(name="psT", bufs=4, space="PSUM"))
            psSc_pool = ctx.enter_context(
                tc.tile_pool(name="psSc", bufs=1, space="PSUM"))
            psAbc_pool = ctx.enter_context(
                tc.tile_pool(name="psAbc", bufs=1, space="PSUM"))
            psPre_pool = ctx.enter_context(
                tc.tile_pool(name="psPre", bufs=2, space="PSUM"))

            # ---- DMA order: enc8[0], first weight quarters, smalls, rest --
            def load_enc8(b):
                t8 = enc8_pool.tile([128, KH * S], fp8, tag="e8", name="e8")
                nc.sync.dma_start(t8[:], enc8_d[b])
                return t8

            def load_enc16(b):
                t16 = enc16_pool.tile([128, KH * S], bf16, tag="e16",
                                      name="e16")
                nc.sync.dma_start(t16[:], enc16_d[b])
                return t16

            e8_tiles = {}
            e16_tiles = {}

            # interleave w1h/w1e chunks: each chunk-pair unlocks more ft
            # columns of both the preamble and the b0 main matmul, so tanh
            # (the ACT critical path) starts ~6us in. Tiny preamble inputs
            # and the first weight pair go ahead of the bulky enc8[0].
            w1e8_sb = consts.tile([128, KF * CHW], fp8)
            w1h8_sb = consts.tile([128, KF * CHW], fp8)
            hidT_32 = consts.tile([128, KH * BL], f32)
            b_attnT_32 = consts.tile([128, KF], f32)
            w2rep_32 = consts.tile([128, KF * BL], f32)
            e8_tiles[0] = load_enc8(0)
            for q in range(NQ):
                nc.sync.dma_start(w1h8_sb[:, q * QW:(q + 1) * QW],
                                  w1h8_d[:, q * QW:(q + 1) * QW])
                if q == 0:
                    nc.sync.dma_start(hidT_32[:], hidT_d[:])
                    nc.sync.dma_start(b_attnT_32[:], b_attnT_d[:])
                nc.sync.dma_start(w1e8_sb[:, q * QW:(q + 1) * QW],
                                  w1e8_d[:, q * QW:(q + 1) * QW])
                if q == 0:
                    nc.sync.dma_start(w2rep_32[:], w2rep_d[:])
                if q == NQ - 3:
                    e8_tiles[1] = load_enc8(1)

            hid8 = consts.tile([128, KH * BL], fp8)
            nc.vector.tensor_copy(hid8[:], hidT_32[:])
            w2rep_sb = consts.tile([128, KF * BL], bf16)
            nc.vector.tensor_copy(w2rep_sb[:], w2rep_32[:])
            ones_row = consts.tile([1, 128], bf16)
            nc.vector.memset(ones_row[:], 1.0)
            # warm the ACT function table (Tanh/Exp set) during the DMA fill
            act_warm = consts.tile([1, 1], bf16)
            nc.scalar.activation(act_warm[:], ones_row[:, 0:1], AF.Tanh)

            e16_tiles[0] = load_enc16(0)
            b_combT_32 = consts.tile([128, KO], f32)
            nc.sync.dma_start(b_combT_32[:], b_combT_d[:])
            decT_32 = consts.tile([128, KH * BL], f32)
            nc.sync.dma_start(decT_32[:], decT_d[:])
            decT_sb = consts.tile([128, KH * BL], bf16)
            nc.vector.tensor_copy(decT_sb[:], decT_32[:])

            appT_sb = consts.tile([128, KH * BL], f32)
            appT_bf = consts.tile([128, KH * BL], bf16)
            # wct_sb[:, kc*H + j] = Wc^T[kc*128+p, j]
            wct_sb = consts.tile([128, KF * H], bf16)

            w1h8_r = w1h8_sb.rearrange("p (t k f) -> p t k f", k=KH, f=128)
            w1e8_r = w1e8_sb.rearrange("p (t k f) -> p t k f", k=KH, f=128)
            hid8_r = hid8.rearrange("p (k b) -> p k b", b=BL)

            # ---- preamble: hidbT[f, b] = (hidden @ W1h.T + b_attn)^T ----
            hidbT_sb = consts.tile([128, KF * BL], f32)
            for ft in range(KF):
                ph = psPre_pool.tile([128, BL], f32, tag="pre", name="ph")
                for kp in range(KP):
                    nc.tensor.matmul(
                        ph[:],
                        w1h8_r[:, ft, 2 * kp:2 * kp + 2, :],
                        hid8_r[:, 2 * kp:2 * kp + 2, :],
                        start=(kp == 0), stop=(kp == KP - 1),
                        perf_mode=PM.DoubleRow)
                # hidbT = psum * 2^-9 + b_attn[f]
                nc.vector.tensor_scalar(
                    out=hidbT_sb[:, ft * BL:(ft + 1) * BL],
                    in0=ph[:],
                    scalar1=1.0 / WSCALE,
                    scalar2=b_attnT_32[:, ft:ft + 1],
                    op0=ALU.mult, op1=ALU.add)

            outT_sb = consts.tile([128, KO * BL], f32)
            if stage < 5:
                nc.vector.memset(appT_sb[:], 0.0)
            if stage < 6:
                nc.vector.memset(outT_sb[:], 0.0)

            # ---- main loop over local batch rows ----
            for b in range(BL if stage >= 5 else (nb if stage >= 2 else 0)):
                if b + 2 < BL:
                    e8_tiles[b + 2] = load_enc8(b + 2)
                if b + 1 < BL:
                    e16_tiles[b + 1] = load_enc16(b + 1)
                # spread the combine-weight prefetch over the tail iterations
                if 3 <= b < 7:
                    q = b - 3
                    nc.sync.dma_start(
                        wct_sb[:, q * 4 * H:(q + 1) * 4 * H]
                        .rearrange("p (a h) -> p a h", a=4),
                        wct[q * 512:(q + 1) * 512, :]
                        .rearrange("(a p) h -> p a h", p=128))
                et8 = e8_tiles.pop(b)
                et16 = e16_tiles.pop(b)
                et8_r = et8.rearrange("p (k s) -> p k s", s=S)
                et16_r = et16.rearrange("p (k s) -> p k s", s=S)

                psc = psSc_pool.tile([BL, S], f32, tag="psc", name="psc")
                for ft in range(KF):
                    pT = psT_pool.tile([128, S], f32, tag="pT", name="pT")
                    for kp in range(KP):
                        nc.tensor.matmul(
                            pT[:],
                            w1e8_r[:, ft, 2 * kp:2 * kp + 2, :],
                            et8_r[:, 2 * kp:2 * kp + 2, :],
                            start=(kp == 0), stop=(kp == KP - 1),
                            perf_mode=PM.DoubleRow)
                    t = th_pool.tile([128, S], bf16, tag="tanh", name="tanh")
                    nc.scalar.activation(
                        t[:], pT[:], AF.Tanh,
                        bias=hidbT_sb[:, ft * BL + b: ft * BL + b + 1],
                        scale=1.0 / WSCALE)
                    if stage >= 3:
                        nc.tensor.matmul(
                            psc[:],
                            w2rep_sb[:, ft * BL:(ft + 1) * BL],
                            t[:],
                            start=(ft == 0), stop=(ft == KF - 1))

                if stage < 3:
                    continue
                # softmax over s on partition 0 (psc rows are replicas).
                # |scores| <= sum|w2|*1 < 46 so exp() cannot overflow fp32:
                # skip the usual max-subtraction pass entirely.
                attn = small_pool.tile([1, S], bf16, tag="attn", name="attn")
                sumexp = small_pool.tile([1, 1], f32, tag="sumexp",
                                         name="sumexp")
                nc.scalar.activation(attn[:], psc[0:1, :], AF.Exp,
                                     bias=0.0, scale=1.0,
                                     accum_out=sumexp[:])
                recip = small_pool.tile([1, 1], f32, tag="recip", name="recip")
                nc.vector.reciprocal(recip[:], sumexp[:])
                recip_rep = small_pool.tile([1, 128], bf16, tag="rrep",
                                            name="rrep")
                nc.vector.tensor_scalar_mul(recip_rep[:], ones_row[:],
                                            recip[:])

                # normalized attention row broadcast to 128 partitions via a
                # K=1 matmul: abc[p, s] = recip * attn[s]. DVE ISA ops cannot
                # read PSUM on HW (device crash), so drain to SBUF bf16.
                abc = psAbc_pool.tile([128, S], f32, tag="abc", name="abc")
                nc.tensor.matmul(abc[:], recip_rep[:], attn[:],
                                 start=True, stop=True)
                abc_sb = abc_pool.tile([128, S], bf16, tag="abcs",
                                       name="abcs")
                nc.vector.tensor_copy(abc_sb[:], abc[:])

                if stage < 4:
                    continue
                # applied^T[h, b] += sum_s enc16[h, s] * abc[h, s].
                # NOTE: tensor_tensor_reduce (fused) crashes the device
                # (NRT_EXEC_UNIT_UNRECOVERABLE) -> use the plain
                # TensorTensor + TensorReduce pair, both HW-proven.
                for kc in range(KH):
                    scr = scr_pool.tile([128, S], bf16, tag="scr", name="scr")
                    nc.vector.tensor_tensor(
                        out=scr[:], in0=et16_r[:, kc, :], in1=abc_sb[:],
                        op=ALU.mult)
                    nc.vector.reduce_sum(
                        appT_sb[:, kc * BL + b: kc * BL + b + 1],
                        scr[:], axis=AX.X)
                    if b == BL - 1:
                        # last row: drain each chunk on idle GPSIMD right
                        # after its ttr so the combine app-terms can start
                        # before the whole DVE chain finishes.
                        nc.gpsimd.tensor_copy(
                            appT_bf[:, kc * BL + b: kc * BL + b + 1],
                            appT_sb[:, kc * BL + b: kc * BL + b + 1])
                if b < BL - 1:
                    nc.vector.tensor_copy(
                        appT_bf.rearrange("p (k b) -> p k b", b=BL)[:, :, b],
                        appT_sb.rearrange("p (k b) -> p k b", b=BL)[:, :, b])

            # ---- combine: outT[hc, b] = tanh(Wc^T-chunk . cat^T + b_comb) --
            for ho in range(KO if stage >= 6 else 0):
                po = psT_pool.tile([128, 512], f32, tag="pT", name="po")
                for kc in range(2 * KH):
                    if kc < KH:
                        lhs = decT_sb[:, kc * BL:(kc + 1) * BL]
                    else:
                        lhs = appT_bf[:, (kc - KH) * BL:(kc - KH + 1) * BL]
                    nc.tensor.matmul(
                        po[:, 0:BL],
                        wct_sb[:, kc * H + ho * 128: kc * H + ho * 128 + 128],
                        lhs,
                        start=(kc == 0), stop=(kc == 2 * KH - 1))
                nc.scalar.activation(
                    outT_sb[:, ho * BL:(ho + 1) * BL], po[:, 0:BL],
                    AF.Tanh, bias=b_combT_32[:, ho:ho + 1])
            HF = KO * BL // 2
            nc.sync.dma_start(appT_d[:], appT_sb[:])
            nc.sync.dma_start(outT_d[:, 0:HF], outT_sb[:, 0:HF])
            nc.sync.dma_start(outT_d[:, HF:], outT_sb[:, HF:])

    nc.compile()
    return nc


def _get_nc():
    if "nc" not in _CACHE:
        _CACHE["nc"] = _build()
    return _CACHE["nc"]


def _swiz_kb(a):
    """[K*128, BL] -> [128, K*BL]: out[p, k*BL+b] = a[k*128+p, b]."""
    k = a.shape[0] // 128
    return np.ascontiguousarray(
        a.reshape(k, 128, -1).transpose(1, 0, 2).reshape(128, -1))


def make_in_maps(inputs):
    import ml_dtypes
    bf = ml_dtypes.bfloat16
    f8 = ml_dtypes.float8_e4m3fn

    inp = {k: np.asarray(v, dtype=np.float32) for k, v in inputs.items()}
    hidden = inp["hidden"]
    decoder_out = inp["decoder_out"]
    encoder_states = inp["encoder_states"]
    W_attn = inp["W_attn"]
    b_attn = inp["b_attn"]
    W_attn2 = inp["W_attn2"]
    W_comb = inp["W_comb"]
    b_comb = inp["b_comb"]
    # b_attn2 shifts every score equally -> softmax-invariant, unused.

    wat = np.ascontiguousarray(W_attn.T)          # [F, F]

    def w1_ftmajor(a):
        # [H, F] -> [128, KF*KH*128]: [p, ft, kc, j] = a[kc*128+p, ft*128+j]
        return np.ascontiguousarray(
            a.reshape(KH, 128, KF, 128).transpose(1, 2, 0, 3)
            .reshape(128, KF * KH * 128))

    sc = np.float32(WSCALE)
    w1h8 = w1_ftmajor(wat[:H] * sc).astype(f8)
    w1e8 = w1_ftmajor(wat[H:] * sc).astype(f8)
    wct = np.ascontiguousarray(W_comb.T).astype(bf)
    w2rep = _swiz_kb(np.repeat(W_attn2.reshape(F, 1), BL, axis=1))
    hidTs = _swiz_kb(np.ascontiguousarray(hidden.T))[
        :, :].reshape(128, KH, NCORES, BL)
    decTs = _swiz_kb(np.ascontiguousarray(decoder_out.T)).reshape(
        128, KH, NCORES, BL)
    b_attnT = np.ascontiguousarray(b_attn.reshape(KF, 128).T)   # [128, KF]
    b_combT = np.ascontiguousarray(b_comb.reshape(KO, 128).T)   # [128, KO]

    in_maps = []
    for c in range(NCORES):
        sl = slice(c * BL, (c + 1) * BL)
        # [S, BL, H] -> [BL, H, S] -> per-b partition-major [BL, 128, KH*S]
        enc_t = np.ascontiguousarray(
            encoder_states[:, sl, :].transpose(1, 2, 0))
        enc_pm = np.ascontiguousarray(
            enc_t.reshape(BL, KH, 128, S).transpose(0, 2, 1, 3)
            .reshape(BL, 128, KH * S))
        in_maps.append({
            "enc8": enc_pm.astype(f8),
            "enc16": enc_pm.astype(bf),
            "w1e8": w1e8,
            "w1h8": w1h8,
            "wct": wct,
            "hidTs": np.ascontiguousarray(hidTs[:, :, c, :]).reshape(
                128, KH * BL),
            "decTs": np.ascontiguousarray(decTs[:, :, c, :]).reshape(
                128, KH * BL),
            "w2reps": w2rep,
            "b_attnT": b_attnT,
            "b_combT": b_combT,
        })
    return in_maps


def _unswiz(a, k):
    """[128, K*BL] -> [BL, K*128]: out[b, kc*128+p] = a[p, kc*BL+b]."""
    return np.ascontiguousarray(
        a.reshape(128, k, BL).transpose(2, 1, 0).reshape(BL, k * 128))


def kernel(**inputs):
    from concourse.bass_utils import run_bass_kernel_spmd

    in_maps = make_in_maps(inputs)
    nc = _get_nc()
    res = run_bass_kernel_spmd(nc, in_maps, list(range(NCORES)))
    out = np.concatenate(
        [_unswiz(np.asarray(res.results[c]["outT"], np.float32), KO)
         for c in range(NCORES)], axis=0)
    applied = np.concatenate(
        [_unswiz(np.asarray(res.results[c]["appliedT"], np.float32), KH)
         for c in range(NCORES)], axis=0)
    return out.astype(np.float32), applied.astype(np.float32)
